# revision 32
# baseline (speedup 1.0000x reference)
"""Distributed Trainium2 kernel for nn_Attention_54795192762650.

GQA attention block with the reference's "scrambled" row-major head
reshapes. 8 NeuronCores: data-parallel over batch (2) x tensor-parallel
over kv-head pairs (4). Because the reference reshapes mix the token and
channel axes, a head's Q slab depends on only 64 token-rows of x but ALL
columns of W_q — so x (token rows) is sharded per core and the weights
are replicated.

Collective-free design: each core computes the output projection of its
OWN 512 channels against ALL 2048 W_out columns (same FLOPs as a column
shard) and returns a partial y[2048, 2048]; the host sums the 4 partials
per batch. This removes the per-quarter AllGather entirely — no
cross-core rendezvous, so the ~100us SPMD core-start skew never stalls
any core's pipeline and every core's span is its own work.

Per core (b = cid//4, c = cid%4, kv heads {2c, 2c+1}):
  - QKV projection of the core's token rows (bf16 matmuls, fp32 PSUM);
    all DMA inputs host-prearranged so each load is a dense block
  - layout shuffles to [d, token] / [j, d] forms; K shuffle + V DRAM
    round-trip run during the Q projection (V on the qAct HWDGE queue)
  - scores S^T[j, l] per head pair via tile_position row-split (K_c=64)
  - causal tile skipping; exp on ScalarE; PV matmul against V padded
    with 64 ones-columns so softmax denominators arrive pre-broadcast
  - per l-quarter: normalize O^T on DVE into otn, then the partial
    output projection (otn stationary, W_out rows for this core's
    channels x all 2048 columns) streams straight out of SBUF

Host side only shards/concats/sums (plus dtype casts and layout
rearranges of the weight blocks).
"""

import sys

import numpy as np

if "/opt/trn_rl_repo" not in sys.path:
    sys.path.insert(0, "/opt/trn_rl_repo")

import ml_dtypes

B, L, D, HD = 2, 2048, 2048, 64
NKV, NG, NH = 8, 4, 32
P = 128
FD = 512          # matmul moving free dim (one fp32 PSUM bank)
KT = D // P       # 16 contraction tiles
NEG = np.float32(-8e9)  # 8 * (-1e9); exp((s+NEG)/8) == 0 in fp32

_NC_CACHE = {}


def _build(causal: bool):
    import concourse.bacc as bacc
    import concourse.tile as tile
    from concourse import mybir

    f32 = mybir.dt.float32
    b16 = mybir.dt.bfloat16
    Exp = mybir.ActivationFunctionType.Exp
    mult = mybir.AluOpType.mult

    nc = bacc.Bacc("TRN2", target_bir_lowering=False, debug=False, num_devices=8)

    xq = nc.dram_tensor("xq", [P, KT, FD], b16, kind="ExternalInput")
    xkv = nc.dram_tensor("xkv", [P, KT, FD], b16, kind="ExternalInput")
    wq = nc.dram_tensor("wq", [4 * KT, P, FD], b16, kind="ExternalInput")
    wk = nc.dram_tensor("wk", [D, FD], b16, kind="ExternalInput")
    wv = nc.dram_tensor("wv", [D, FD], b16, kind="ExternalInput")
    wo = nc.dram_tensor("wo", [P, NG, D], b16, kind="ExternalInput")
    mtmpl = nc.dram_tensor("mtmpl", [P, 896], b16, kind="ExternalInput")
    eye = nc.dram_tensor("eye", [P, P], b16, kind="ExternalInput")
    if not causal:
        mt8 = nc.dram_tensor("mt8", [P, KT, L], b16, kind="ExternalInput")
    out = nc.dram_tensor("out", [L, D], f32, kind="ExternalOutput")

    with tile.TileContext(nc) as tc:
        with tc.tile_pool(name="pres", bufs=1) as pres, \
             tc.tile_pool(name="wpool", bufs=8) as wpool, \
             tc.tile_pool(name="pearly", bufs=1) as pearly, \
             tc.tile_pool(name="dram", bufs=1, space="DRAM") as dram:
            mt_sb = pres.tile([P, 896], b16, name="mt_sb", tag="mt_sb")
            nc.sync.dma_start(mt_sb[:], mtmpl[:])
            eye_sb = pres.tile([P, P], b16, name="eye_sb", tag="eye_sb")
            nc.sync.dma_start(eye_sb[:], eye[:])
            # otn[hd*64+d, g, l]: normalized O^T for head (g, hd) — the
            # out-projection's stationary operand, channels on partitions.
            otn = pres.tile([P, NG, L], b16, name="otn", tag="otn")
            wo_sb = pres.tile([P, NG, D], b16, name="wo_sb", tag="wo_sb")
            nc.scalar.dma_start(wo_sb[:], wo[:])
            if not causal:
                mt8_sb = pres.tile([P, KT, L], b16, name="mt8_sb", tag="mt8_sb")
                nc.scalar.dma_start(mt8_sb[:], mt8[:])

            slabk = [pearly.tile([P, 2, 512], b16, name=f"slabk{h}", tag=f"slabk{h}")
                     for h in range(2)]
            slabv = [pearly.tile([P, 2, 512], b16, name=f"slabv{h}", tag=f"slabv{h}")
                     for h in range(2)]
            slabq = [pearly.tile([P, L], b16, name=f"slabq{g}", tag=f"slabq{g}")
                     for g in range(NG)]
            kt_sb = pearly.tile([P, L], b16, name="kt_sb", tag="kt_sb")
            v_sb = [pearly.tile([P, KT, P], b16, name=f"v_sb{h}", tag=f"v_sb{h}")
                    for h in range(2)]
            # per-(g, l-half) tiles so attention quarters 0/1 only depend on
            # their own block moves (Tile tracks dependencies per tile)
            qt_sb = [[pearly.tile([P, 2 * FD], b16, name=f"qt_sb{g}_{tl}",
                                  tag=f"qt_sb{g}_{tl}") for tl in range(2)]
                     for g in range(NG)]
            qst = [pearly.tile([P, L], b16, name=f"qst{g}", tag=f"qst{g}")
                   for g in range(NG)]

            # HAM warm-up: keep the PE busy during the initial input DMAs so
            # the projection matmuls start at the full 2.4 GHz clock. Sourced
            # from a memset tile so no DMA gates the first matmul.
            warm_src = pearly.tile([P, FD], b16, name="warm_src", tag="warm_src")
            nc.vector.memset(warm_src[:], 0.5)
            with tc.tile_pool(name="pswarm", bufs=1, space="PSUM") as pswarm:
                warm_ps = pswarm.tile([P, FD], f32, name="warm_ps", tag="warm")
                for _ in range(36):
                    nc.tensor.matmul(warm_ps[:], warm_src[:, 0:P], warm_src[:],
                                     start=True, stop=True)

            # ------------- phase A: projections + shuffles -------------
            with tc.tile_pool(name="xpool", bufs=1) as xpool:
                # interleave per-kt x/weight loads so the first KV matmul only
                # waits for one kt's worth of data
                xkv_sb = xpool.tile([P, KT, 512], b16, name="xkv_sb", tag="xkv_sb")
                wk_ts, wv_ts = [], []
                for kt in range(KT):
                    nc.sync.dma_start(xkv_sb[:, kt, :], xkv[:, kt, :])
                    wk_t = wpool.tile([P, FD], b16, name="wk_t", tag="wk_t", bufs=KT)
                    nc.sync.dma_start(wk_t[:], wk[kt * P:(kt + 1) * P, :])
                    wk_ts.append(wk_t)
                    wv_t = wpool.tile([P, FD], b16, name="wv_t", tag="wv_t", bufs=KT)
                    nc.sync.dma_start(wv_t[:], wv[kt * P:(kt + 1) * P, :])
                    wv_ts.append(wv_t)
                xq_sb = xpool.tile([P, KT, 512], b16, name="xq_sb", tag="xq_sb")

                with tc.tile_pool(name="pskv", bufs=8, space="PSUM") as pskv:
                    pk = {}
                    for hb in range(2):
                        for th in range(2):
                            pk[("k", hb, th)] = pskv.tile([P, FD], f32,
                                                          name=f"pk{hb}{th}", tag="pj")
                            pk[("v", hb, th)] = pskv.tile([P, FD], f32,
                                                          name=f"pv{hb}{th}", tag="pj")
                    for kt in range(KT):
                        for hb in range(2):
                            for th in range(2):
                                lhsT = xkv_sb[:, kt, hb * 256 + th * P: hb * 256 + (th + 1) * P]
                                nc.tensor.matmul(pk[("k", hb, th)][:], lhsT, wk_ts[kt][:],
                                                 start=(kt == 0), stop=(kt == KT - 1))
                                nc.tensor.matmul(pk[("v", hb, th)][:], lhsT, wv_ts[kt][:],
                                                 start=(kt == 0), stop=(kt == KT - 1))
                    for hb in range(2):
                        for th in range(2):
                            nc.scalar.copy(slabk[hb][:, th, :], pk[("k", hb, th)][:])
                            nc.vector.tensor_copy(slabv[hb][:, th, :],
                                                  pk[("v", hb, th)][:])

                # K shuffle (DVE) + V DRAM round-trip (qAct HWDGE queue),
                # both overlap the Q projection below.
                # KT_sb[64*hb + d, j] = K_hb[j, d],  j = t*8 + u
                for hb in range(2):
                    for th in range(2):
                        kst = pearly.tile([P, FD], b16, name="kst", tag="kst", bufs=2)
                        nc.vector.transpose(kst[:], slabk[hb][:, th, :])
                        for tl in range(4):
                            for be in range(2):
                                src = kst[32 * tl:32 * tl + 32, :].rearrange(
                                    "p (u bd) -> p bd u", u=8)[:, 32 * be:32 * be + 32, :]
                                o_base = th * 1024 + tl * 256
                                dst = kt_sb[64 * hb + 32 * be: 64 * hb + 32 * be + 32,
                                            o_base:o_base + 256].rearrange(
                                    "p (tt u) -> p tt u", u=8)
                                nc.vector.tensor_copy(dst, src)
                # V via DRAM round trip; ones-columns 64:128 make the PV matmul
                # emit softmax denominators pre-broadcast on PSUM rows 64:128.
                for hb in range(2):
                    vsc = dram.tile([256, 512], b16, name=f"vsc{hb}", tag=f"vsc{hb}")
                    for th in range(2):
                        nc.scalar.dma_start(vsc[th * P:(th + 1) * P, :], slabv[hb][:, th, :])
                    nc.scalar.dma_start(
                        v_sb[hb][:, :, 0:64],
                        vsc.rearrange("(jt tl) (u d) -> (tl u) jt d", tl=16, u=8))
                    nc.vector.memset(v_sb[hb][:, :, 64:128], 1.0)

                with tc.tile_pool(name="psq", bufs=8, space="PSUM") as psq:
                    for cc in range(4):
                        pq = [psq.tile([P, FD], f32, name=f"pq{g}", tag="pq")
                              for g in range(NG)]
                        for kt in range(KT):
                            if cc == 0:
                                nc.sync.dma_start(xq_sb[:, kt, :], xq[:, kt, :])
                            wq_t = wpool.tile([P, FD], b16, name="wq_t",
                                              tag="wq_t", bufs=16)
                            nc.sync.dma_start(wq_t[:], wq[cc * KT + kt])
                            for g in range(NG):
                                lhsT = xq_sb[:, kt, g * P:(g + 1) * P]
                                nc.tensor.matmul(pq[g][:], lhsT, wq_t[:],
                                                 start=(kt == 0), stop=(kt == KT - 1))
                        for g in range(NG):
                            nc.scalar.copy(slabq[g][:, cc * FD:(cc + 1) * FD], pq[g][:])
                            # transpose this chunk now so only the block moves
                            # remain after the projection finishes
                            nc.vector.transpose(qst[g][:, cc * FD:(cc + 1) * FD],
                                                slabq[g][:, cc * FD:(cc + 1) * FD])

            # Q block moves (DVE):
            # QT_sb[g][tl][64*hd + d, l'] = Q_(pair g, hd)[l, d], l = tl*1024+l'.
            # tl=0 is emitted here; tl=1 (needed from quarter 2 on) is emitted
            # after quarter 0 so it doesn't jam the DVE FIFO ahead of the
            # first quarter's normalizes.
            def _qhalf(tl):
                for g in range(NG):
                    for hd in range(2):
                        for be in range(2):
                            src = qst[g][64 * hd + 32 * tl: 64 * hd + 32 * tl + 32,
                                         :].rearrange(
                                "p (u bd) -> p bd u", u=32)[:, 32 * be:32 * be + 32, :]
                            dst = qt_sb[g][tl][64 * hd + 32 * be:
                                               64 * hd + 32 * be + 32,
                                               :].rearrange("p (tt u) -> p tt u", u=32)
                            nc.vector.tensor_copy(dst, src)

            _qhalf(0)

            # ------------- phase C: attention + partial out-projection -------------
            with tc.tile_pool(name="apool", bufs=1) as apool:
                with tc.tile_pool(name="psc", bufs=1, space="PSUM") as psc:
                    for m in range(4):
                        jt_max = 4 * m + 4 if causal else KT
                        for g in range(NG):
                            po = [psc.tile([P, FD], f32, name=f"po{hd}", tag="po", bufs=2)
                                  for hd in range(2)]
                            hist = {}
                            for jt in range(jt_max + 1):
                                if jt < jt_max:
                                    ps = psc.tile([P, 2 * FD], f32, name="ps",
                                                  tag="ps", bufs=3)
                                    e_t = apool.tile([P, 2 * FD], b16, name="e_t",
                                                     tag="e_t", bufs=6)
                                    s_ = jt - 4 * m
                                    strad = causal and s_ >= 0
                                    z = 128 * s_ if strad else 0  # fully-masked prefix
                                    lq = (m % 2) * FD
                                    for hd in range(2):
                                        sl = ps[:, hd * FD + z:(hd + 1) * FD]
                                        pre = False
                                        if strad:
                                            nc.tensor.matmul(
                                                sl, eye_sb[:], mt_sb[:, 384:896 - z],
                                                start=True, stop=False)
                                            pre = True
                                        elif not causal:
                                            nc.tensor.matmul(
                                                sl, eye_sb[:],
                                                mt8_sb[:, jt, m * FD + z:(m + 1) * FD],
                                                start=True, stop=False)
                                            pre = True
                                        nc.tensor.matmul(
                                            sl,
                                            kt_sb[64 * hd:64 * hd + 64, jt * P:(jt + 1) * P],
                                            qt_sb[g][m // 2][64 * hd:64 * hd + 64,
                                                             lq + z:lq + FD],
                                            start=not pre, stop=True,
                                            tile_position=(64 * hd, 0))
                                    if z:
                                        exp_in = ps[:, :].rearrange(
                                            "p (hd l) -> p hd l", hd=2)[:, :, z:]
                                        exp_out = e_t[:, :].rearrange(
                                            "p (hd l) -> p hd l", hd=2)[:, :, z:]
                                        nc.scalar.activation(exp_out, exp_in, Exp,
                                                             scale=0.125)
                                    else:
                                        nc.scalar.activation(e_t[:], ps[:], Exp,
                                                             scale=0.125)
                                    hist[jt] = e_t
                                if jt >= 1:
                                    jp = jt - 1
                                    # masked prefix of straddle tiles is never
                                    # written by exp; trim PV to the live
                                    # columns instead of zero-filling e_t
                                    sp_ = jp - 4 * m
                                    zp = 128 * sp_ if (causal and sp_ >= 0) else 0
                                    for hd in range(2):
                                        nc.tensor.matmul(
                                            po[hd][:, zp:], v_sb[hd][:, jp, :],
                                            hist[jp][:, hd * FD + zp:(hd + 1) * FD],
                                            start=(jp == 0), stop=(jp == jt_max - 1),
                                            skip_group_check=True)
                                    del hist[jp]
                            # copy both po halves out first (base-0 tiles)
                            # so the PSUM pair frees before the reciprocal
                            # chain and the next pair's first PV never stalls
                            nums, dens = [], []
                            for hd in range(2):
                                num = apool.tile([64, FD], f32, name="pnum",
                                                 tag="pnum", bufs=4)
                                nc.vector.tensor_copy(num[:], po[hd][0:64, :])
                                nums.append(num)
                                den = apool.tile([64, FD], f32, name="pden",
                                                 tag="pden", bufs=4)
                                nc.vector.tensor_copy(den[:], po[hd][64:128, :])
                                dens.append(den)
                            for hd in range(2):
                                srec = apool.tile([64, FD], f32, name="srec",
                                                  tag="srec", bufs=3)
                                nc.vector.reciprocal_approx_fast(srec[:], dens[hd][:])
                                nc.vector.tensor_tensor(
                                    otn[64 * hd:64 * hd + 64, g, m * FD:(m + 1) * FD],
                                    nums[hd][:], srec[:], mult)

                        # partial out-projection for this l-quarter: contraction
                        # over this core's 512 channels (4 g-slabs of 128), all
                        # 2048 output columns; purely core-local. Column pairs
                        # accumulate into one 2-bank PSUM tile drawn from the
                        # "ps" rotation (idle during this stretch) so the PE
                        # never waits on the copy-out, and the two co matmuls
                        # per g share the stationary otn slice.
                        for lt in range(4):
                            pys = [psc.tile([P, 2 * FD], f32, name=f"py{cp}",
                                            tag="ps", bufs=3) for cp in range(2)]
                            for g in range(NG):
                                # one stationary otn slice streams all 4
                                # column chunks (2048 moving rows per load)
                                for co in range(4):
                                    nc.tensor.matmul(
                                        pys[co // 2][:, (co % 2) * FD:
                                                     (co % 2 + 1) * FD],
                                        otn[:, g,
                                            m * FD + lt * P: m * FD + (lt + 1) * P],
                                        wo_sb[:, g, co * FD:(co + 1) * FD],
                                        start=(g == 0), stop=(g == NG - 1))
                            for cp in range(2):
                                y_sb = apool.tile([P, 2 * FD], f32, name="y_sb",
                                                  tag="y_sb", bufs=3)
                                nc.vector.tensor_copy(y_sb[:], pys[cp][:])
                                nc.sync.dma_start(
                                    out[(4 * m + lt) * P:(4 * m + lt + 1) * P,
                                        2 * cp * FD:2 * (cp + 1) * FD],
                                    y_sb[:])
                        if m == 1:
                            _qhalf(1)

    nc.compile()
    return nc


def _get_nc(causal: bool):
    if causal not in _NC_CACHE:
        _NC_CACHE[causal] = _build(causal)
    return _NC_CACHE[causal]


def kernel(x, mask, W_qkv, W_out):
    from concourse.bass_utils import run_bass_kernel_spmd

    bf = ml_dtypes.bfloat16
    x = np.asarray(x, dtype=np.float32)
    mask = np.asarray(mask, dtype=np.float32)
    W_qkv = np.asarray(W_qkv, dtype=np.float32)
    W_out = np.asarray(W_out, dtype=np.float32)

    xT = np.ascontiguousarray(x.transpose(0, 2, 1)).astype(bf)  # [B, k, l]
    Wq = W_qkv[:, :2048]
    Wk = np.ascontiguousarray(W_qkv[:, 2048:2560]).astype(bf)
    Wv = np.ascontiguousarray(W_qkv[:, 2560:3072]).astype(bf)
    # wq blocks: [(cc kt), p, c] so each (cc, kt) load is one dense 128KB block
    wq_blk = np.ascontiguousarray(
        Wq.reshape(KT, P, 4, FD).transpose(2, 0, 1, 3).reshape(4 * KT, P, FD)
    ).astype(bf)

    tril = np.tril(np.ones((L, L), dtype=bool))
    expected = np.where(tril, np.float32(0.0), np.float32(-1e9))
    causal = bool(np.array_equal(mask, expected))

    pp = np.arange(P)[:, None]
    qq = np.arange(896)[None, :]
    mtmpl = np.where(pp > qq - 384, NEG, np.float32(0.0)).astype(bf)
    eyem = np.eye(P, dtype=np.float32).astype(bf)

    # W_out rows for core pair c, channels ordered [hb*64+d, g]
    hbd = np.arange(P)
    hb_idx = hbd // 64
    d_idx = hbd % 64

    in_maps = []
    for cid in range(8):
        b, c = divmod(cid, 4)
        h0 = 2 * c
        qrows = np.concatenate(
            [np.arange(64 * (8 * g + h0), 64 * (8 * g + h0) + 128) for g in range(NG)])
        xq_h = np.ascontiguousarray(
            xT[b][:, qrows].reshape(KT, P, FD).transpose(1, 0, 2))
        xkv_h = np.ascontiguousarray(
            xT[b][:, 512 * c:512 * c + 512].reshape(KT, P, FD).transpose(1, 0, 2))
        rows = (np.arange(NG)[None, :] * 512 + (2 * c + hb_idx)[:, None] * 64
                + d_idx[:, None])  # [128, 4] original W_out row ids
        wo_h = np.ascontiguousarray(W_out[rows, :]).astype(bf)  # [128, 4, 2048]
        im = {
            "xq": xq_h, "xkv": xkv_h,
            "wq": wq_blk, "wk": Wk, "wv": Wv, "wo": wo_h,
            "mtmpl": mtmpl, "eye": eyem,
        }
        if not causal:
            im["mt8"] = np.ascontiguousarray(
                (8.0 * mask.T).astype(bf).reshape(KT, P, L).transpose(1, 0, 2))
        in_maps.append(im)

    nc = _get_nc(causal)
    res = run_bass_kernel_spmd(nc, in_maps, list(range(8)))
    outp = np.empty((B, L, D), dtype=np.float32)
    for b in range(B):
        acc = res.results[4 * b]["out"].astype(np.float32)
        for c in range(1, 4):
            acc = acc + res.results[4 * b + c]["out"]
        outp[b] = acc
    return outp
